# revision 13
# baseline (speedup 1.0000x reference)
"""DKBATNet GNN message-passing kernel for 8 Trainium2 NeuronCores.

Approach:
- Decompose the per-edge linear layer: c = U[src] + V[dst] + T[etype] (+bias
  folded into T), with per-node tables U (+ attention scalars au/av packed in
  the same 192-col rows) built by dense f32r matmuls on-device, and per-type
  tables T host-computed from g/W.
- Edges sorted by key0 and by key1, sharded across 8 cores by the sort key's
  node range, so every segment-sum (softmax denominators, aggregations) is
  core-local; only [N,2] denominator tables and the [N,128] inter-layer
  activations are AllGathered.
- Per-edge rows fetched with dma_gather (768B rows, lo/hi split for int16
  index range); per-128-edge-chunk one-hot matrices (iota + is_equal) turn
  segment-sums and sorted-key expansions into tensor-engine matmuls.
"""

from contextlib import ExitStack

import numpy as np

import concourse.bacc as bacc
import concourse.bass as bass
import concourse.mybir as mybir
import concourse.tile as tile
from concourse.bass_utils import run_bass_kernel_spmd
from concourse.masks import make_identity
from concourse.vector_clock import ScopedClock

def _install_ntff_shim():
    """Register antenv.axon_hooks so BASS_TRACE=1 profiling works here."""
    import sys
    import types

    name = "antenv.axon_hooks"
    if name in sys.modules:
        return
    mod = types.ModuleType(name)
    mod._HOOK = None
    mod.set_axon_ntff_profile_hook = lambda h: setattr(mod, "_HOOK", h)
    mod.get_axon_ntff_profile_hook = lambda: mod._HOOK
    try:
        if "/root/.axon_site" not in sys.path:
            sys.path.insert(0, "/root/.axon_site")
        from trn_agent_boot.trn_boot import _ntff_profile_via_ctypes

        mod._HOOK = _ntff_profile_via_ctypes("/opt/axon/libaxon_pjrt.so")
    except Exception:
        pass
    sys.modules[name] = mod
    try:
        import antenv

        antenv.axon_hooks = mod
    except ImportError:
        pass


_install_ntff_shim()

F32 = mybir.dt.float32
F32R = mybir.dt.float32r
I16 = mybir.dt.int16
AF = mybir.ActivationFunctionType
ALU = mybir.AluOpType

NCORES = 8
P = 128
HI_SPLIT = 32768
NEG_SLOPE = 0.01
ALPHA_MIX = 0.5
EPS = 1e-12
NT = 192  # big table row width (floats)
MAXC = 16  # max chunks (of 128 edges) per node tile


# ---------------------------------------------------------------- tile patch
def _patched_drain_and_barrier(self, tick_clock, wait_clock):
    nc = self.nc
    carrier = nc.sync.nop(nofuse=True, hint="drain_wait_carrier")
    wait_clock.add_sem_waits(carrier.ins, ScopedClock({None: tick_clock.global_clock}))
    si = carrier.ins.sync_info
    waits = list(si.on_wait) if si is not None else []
    if len(waits) > 1:
        si.on_wait = [waits[0]]
        for w in waits[1:]:
            n = nc.sync.nop(nofuse=True, hint="drain_wait_split")
            nsi = n.ins.sync_info
            if nsi is None:
                n.ins.sync_info = mybir.SyncInfo(on_wait=[w], on_update=[])
            else:
                nsi.on_wait = [w]
    nc.sync.drain()
    nc.all_engine_barrier()
    assert self.sems is not None
    popped = nc._tile_sem_poison_stack.pop()
    assert popped is self._sem_poison
    nc.clear_and_free_semaphores(list(self.sems.allocated().values()))
    nc.all_engine_barrier()


tile.TileContext._drain_and_barrier = _patched_drain_and_barrier


def split_excess_waits(nc, max_waits=1):
    for f in nc.m.functions:
        for bb in f.blocks:
            insts = bb.instructions
            if not any(
                i.sync_info is not None and len(i.sync_info.on_wait) > max_waits
                for i in insts
            ):
                continue
            new = []
            for inst in insts:
                si = inst.sync_info
                if si is not None and len(si.on_wait) > max_waits:
                    waits = list(si.on_wait)
                    excess, keep = waits[:-max_waits], waits[-max_waits:]
                    for j in range(0, len(excess), max_waits):
                        nop = mybir.InstNoOp(
                            name=f"wsplit_{inst.name}_{j}", ins=[], outs=[]
                        )
                        nop.engine = inst.engine
                        nop.sync_info = mybir.SyncInfo(
                            on_wait=excess[j : j + max_waits], on_update=[]
                        )
                        new.append(nop)
                    si.on_wait = keep
                new.append(inst)
            bb.instructions = new


# ---------------------------------------------------------------- host prep
def _wrap16_into(dst, col0, idx):
    n = len(idx)
    w = idx.reshape(n // 16, 16).T.astype(np.int16)
    dst[:, col0 : col0 + n // 16] = np.tile(w, (8, 1))


class EdgeOrder:
    """Edges sorted by `key`, sharded by key range, tiled by 128 sort-key
    nodes; each tile's edges split lo/hi by far key, 128-padded, with
    uniform (max-over-cores) section sizes so the SPMD program is shared."""

    def __init__(self, key, far, et, N, ns, ntiles):
        self.N, self.ns, self.ntiles = N, ns, ntiles
        perm = np.argsort(key, kind="stable")
        key, far, et = key[perm], far[perm], et[perm]
        self.sections = []
        for k in range(NCORES):
            tiles = []
            for t in range(ntiles):
                base = k * ns + t * P
                top = min((k + 1) * ns, base + P)
                s = np.searchsorted(key, base)
                e = np.searchsorted(key, top)
                kk, ff, ee = key[s:e], far[s:e], et[s:e]
                m = ff < HI_SPLIT
                tiles.append(
                    (
                        (kk[m] - base, ff[m], ee[m]),
                        (kk[~m] - base, ff[~m] - HI_SPLIT, ee[~m]),
                    )
                )
            self.sections.append(tiles)

        def up128(v):
            return max(-(-v // P) * P, P)

        self.n_lo = [
            up128(max(len(self.sections[k][t][0][0]) for k in range(NCORES)))
            for t in range(ntiles)
        ]
        self.n_hi = [
            up128(max(len(self.sections[k][t][1][0]) for k in range(NCORES)))
            for t in range(ntiles)
        ]
        self.n_tile = [self.n_lo[t] + self.n_hi[t] for t in range(ntiles)]
        self.S = int(sum(self.n_tile))
        self.off = np.concatenate([[0], np.cumsum(self.n_tile)]).astype(int)
        assert max(self.n_tile) <= MAXC * P, max(self.n_tile)

    def core_arrays(self, k):
        S = self.S
        idx_far = np.zeros((P, S // 16), np.int16)
        idx_et = np.zeros((P, S // 16), np.int16)
        keyloc = np.full(S, -1.0, np.float32)
        for t in range(self.ntiles):
            off = int(self.off[t])
            (klo, flo, elo), (khi, fhi, ehi) = self.sections[k][t]
            nlo, nhi = self.n_lo[t], self.n_hi[t]

            def padded(a, n):
                out = np.zeros(n, np.int64)
                out[: len(a)] = a
                return out

            _wrap16_into(idx_far, off // 16, padded(flo, nlo))
            _wrap16_into(idx_far, (off + nlo) // 16, padded(fhi, nhi))
            _wrap16_into(idx_et, off // 16, padded(elo, nlo))
            _wrap16_into(idx_et, (off + nlo) // 16, padded(ehi, nhi))
            keyloc[off : off + len(klo)] = klo
            keyloc[off + nlo : off + nlo + len(khi)] = khi
        keyloc_col = np.ascontiguousarray(keyloc.reshape(S // P, P).T)
        keyrow = keyloc.reshape(1, S)
        return idx_far, idx_et, keyloc_col, keyrow


def _head_fold(Wm, av, H):
    Dn = Wm.shape[0] // H
    return np.stack(
        [
            Wm[h * Dn : (h + 1) * Dn, :].T @ av[h * Dn : (h + 1) * Dn]
            for h in range(H)
        ],
        1,
    ).astype(np.float32)


def _prep_host(x, g, edge_idx, edge_type, W):
    N, Dx = x.shape
    R, Dg = g.shape
    ns = N // NCORES
    ntiles = -(-ns // P)
    H = 2

    key0 = np.asarray(edge_idx[0], np.int64)
    key1 = np.asarray(edge_idx[1], np.int64)
    et = np.asarray(edge_type, np.int64)

    xn = (x / np.maximum(np.linalg.norm(x, axis=1, keepdims=True), EPS)).astype(
        np.float32
    )
    xn_aug = np.concatenate([xn, np.ones((N, 1), np.float32)], 1)

    def build_weights(Wi, bi, ai, Wo, bo, ao, d_in):
        Wai, Wbi, Wgi = Wi[:, :d_in], Wi[:, d_in : 2 * d_in], Wi[:, 2 * d_in :]
        Wao, Wbo, Wgo = Wo[:, :d_in], Wo[:, d_in : 2 * d_in], Wo[:, 2 * d_in :]
        avi, avo = ai[0].reshape(-1), ao[0].reshape(-1)
        wa_i = _head_fold(Wai, avi, H)
        wb_i = _head_fold(Wbi, avi, H)
        wa_o = _head_fold(Wao, avo, H)
        wb_o = _head_fold(Wbo, avo, H)
        # [U_i(128)|au_i(2)|av_o(2) | U_o(128)|au_o(2)|av_i(2)] = 264 cols
        Wf_a = np.concatenate(
            [Wai.T, wa_i, wb_o, Wao.T, wa_o, wb_i], 1
        ).astype(np.float32)
        Wf_b = np.concatenate([Wbi.T, Wbo.T], 1).astype(np.float32)
        Ti = (g @ Wgi.T + bi).astype(np.float32)
        To = (g @ Wgo.T + bo).astype(np.float32)
        Dn = Wai.shape[0] // H
        ati = np.stack(
            [Ti[:, h * Dn : (h + 1) * Dn] @ avi[h * Dn : (h + 1) * Dn]
             for h in range(H)], 1).astype(np.float32)
        ato = np.stack(
            [To[:, h * Dn : (h + 1) * Dn] @ avo[h * Dn : (h + 1) * Dn]
             for h in range(H)], 1).astype(np.float32)
        Tti = np.zeros((R, NT), np.float32)
        Tti[:, :128], Tti[:, 128:130], Tti[:, 130:132] = Ti, ati, ato
        Tto = np.zeros((R, NT), np.float32)
        Tto[:, :128], Tto[:, 128:130] = To, ato
        Sat = np.zeros((R, 64), np.float32)
        Sat[:, 0:2] = ati
        return Wf_a, Wf_b, Sat, Tti, Tto, wb_i

    Wf_a1, Wf_b1, Sat1, Tti1, Tto1, wb_i1 = build_weights(
        W["W1i"], W["b1i"], W["a1i"], W["W1o"], W["b1o"], W["a1o"], Dx
    )
    Wf_a2, Wf_b2, Sat2, Tti2, Tto2, _ = build_weights(
        W["W2i"], W["b2i"], W["a2i"], W["W2o"], W["b2o"], W["a2o"], 2 * 64
    )
    Wf_a1_aug = np.zeros((Dx + 1, 264), np.float32)
    Wf_a1_aug[:Dx] = Wf_a1
    Wf_b1_aug = np.zeros((Dx + 1, 320), np.float32)
    Wf_b1_aug[:Dx, :256] = Wf_b1
    Wf_b1_aug[:Dx, 256:320] = W["We"].T
    Wf_b1_aug[Dx, 256:320] = W["be"]

    Sav1 = np.zeros((N, 64), np.float32)
    Sav1[:, 0:2] = xn @ wb_i1

    Wr_aug = np.zeros((Dg + 1, 128), np.float32)
    Wr_aug[:Dg] = W["Wr"].T
    Wr_aug[Dg] = W["br"]
    gT_aug = np.ascontiguousarray(
        np.concatenate([g.T, np.ones((1, R), np.float32)], 0)
    )

    sa = EdgeOrder(key0, key1, et, N, ns, ntiles)  # dn_in + agg_out passes
    sb = EdgeOrder(key1, key0, et, N, ns, ntiles)  # dn_out + agg_in pass

    meta = dict(N=N, R=R, ns=ns, ntiles=ntiles, Dx=Dx, Dg=Dg, sa=sa, sb=sb)

    xnT_aug = np.ascontiguousarray(xn_aug.T)
    in_maps = []
    for k in range(NCORES):
        fa_sa, et_sa, kc_sa, kr_sa = sa.core_arrays(k)
        fa_sb, et_sb, kc_sb, kr_sb = sb.core_arrays(k)
        in_maps.append(dict(
            xnT=xnT_aug,
            xnT_own=np.ascontiguousarray(xn_aug[k * ns : (k + 1) * ns].T),
            Wf_a1=Wf_a1_aug, Wf_b1=Wf_b1_aug, Wf_a2=Wf_a2, Wf_b2=Wf_b2,
            Tti1=Tti1, Tto1=Tto1, Sat1=Sat1,
            Tti2=Tti2, Tto2=Tto2, Sat2=Sat2,
            Sav1=Sav1, Wr=Wr_aug, gT=gT_aug,
            fa_sa=fa_sa, et_sa=et_sa, kc_sa=kc_sa, kr_sa=kr_sa,
            fa_sb=fa_sb, et_sb=et_sb, kc_sb=kc_sb, kr_sb=kr_sb,
        ))
    return meta, in_maps


# ---------------------------------------------------------------- kernel IR
def _build_nc(meta, split=True):
    N, R, ns, ntiles = meta["N"], meta["R"], meta["ns"], meta["ntiles"]
    Dx, Dg = meta["Dx"], meta["Dg"]
    sa, sb = meta["sa"], meta["sb"]

    nc = bacc.Bacc(
        "TRN2", target_bir_lowering=False, debug=False,
        num_devices=NCORES, dynamic_dma_scratch_size=16384,
    )
    dt = nc.dram_tensor

    t_xnT = dt("xnT", [Dx + 1, N], F32, kind="ExternalInput")
    t_xnT_own = dt("xnT_own", [Dx + 1, ns], F32, kind="ExternalInput")
    t_Wf_a1 = dt("Wf_a1", [Dx + 1, 264], F32, kind="ExternalInput")
    t_Wf_b1 = dt("Wf_b1", [Dx + 1, 320], F32, kind="ExternalInput")
    t_Wf_a2 = dt("Wf_a2", [P, 264], F32, kind="ExternalInput")
    t_Wf_b2 = dt("Wf_b2", [P, 256], F32, kind="ExternalInput")
    t_Tt = {(1, "i"): dt("Tti1", [R, NT], F32, kind="ExternalInput"),
            (1, "o"): dt("Tto1", [R, NT], F32, kind="ExternalInput"),
            (2, "i"): dt("Tti2", [R, NT], F32, kind="ExternalInput"),
            (2, "o"): dt("Tto2", [R, NT], F32, kind="ExternalInput")}
    t_Sat = {1: dt("Sat1", [R, 64], F32, kind="ExternalInput"),
             2: dt("Sat2", [R, 64], F32, kind="ExternalInput")}
    t_Sav1 = dt("Sav1", [N, 64], F32, kind="ExternalInput")
    t_Wr = dt("Wr", [Dg + 1, P], F32, kind="ExternalInput")
    t_gT = dt("gT", [Dg + 1, R], F32, kind="ExternalInput")
    t_idx = {}
    for o, eo in (("sa", sa), ("sb", sb)):
        t_idx["fa_" + o] = dt(f"fa_{o}", [P, eo.S // 16], I16, kind="ExternalInput")
        t_idx["et_" + o] = dt(f"et_{o}", [P, eo.S // 16], I16, kind="ExternalInput")
        t_idx["kc_" + o] = dt(f"kc_{o}", [P, eo.S // P], F32, kind="ExternalInput")
        t_idx["kr_" + o] = dt(f"kr_{o}", [1, eo.S], F32, kind="ExternalInput")

    o_h = dt("hprime", [ns, P], F32, kind="ExternalOutput")
    o_gp = dt("gprime", [R, P], F32, kind="ExternalOutput")

    d_Ui = dt("Uti", [N, NT], F32)
    d_Uo = dt("Uto", [N, NT], F32)
    d_Sav = dt("Sav2", [N, 64], F32)
    d_V = dt("Vown", [ns, 256], F32)
    d_hin = dt("hin", [ns, P], F32)
    d_hout = dt("hout", [ns, P], F32)
    d_hT_in = dt("hT_in", [P, ns], F32)
    d_hT = dt("hT_ag", [NCORES * P, ns], F32, addr_space="Shared")
    d_dn = {}
    for l in (1, 2):
        for dd in ("i", "o"):
            d_dn[(l, dd)] = (dt(f"dn_{dd}{l}_in", [ns, 2], F32),
                             dt(f"dn_{dd}{l}_out", [N, 2], F32,
                                addr_space="Shared"))
    RG = [list(range(NCORES))]

    with tile.TileContext(nc) as tc, ExitStack() as ctx:
        const = ctx.enter_context(tc.tile_pool(name="const", bufs=1))
        lhsp = ctx.enter_context(tc.tile_pool(name="lhsp", bufs=3))
        stag = ctx.enter_context(tc.tile_pool(name="stag", bufs=2))
        gathp = ctx.enter_context(tc.tile_pool(name="gathp", bufs=2))
        gathp2 = ctx.enter_context(tc.tile_pool(name="gathp2", bufs=1))
        mpool = ctx.enter_context(tc.tile_pool(name="mpool", bufs=2))
        work = ctx.enter_context(tc.tile_pool(name="work", bufs=2))
        small = ctx.enter_context(tc.tile_pool(name="small", bufs=2))
        own = ctx.enter_context(tc.tile_pool(name="own", bufs=1))
        idxp = ctx.enter_context(tc.tile_pool(name="idxp", bufs=1))
        psum = ctx.enter_context(tc.tile_pool(name="psum", bufs=1, space="PSUM"))
        psagg = psum

        iota_chunk = const.tile([P, MAXC, P], F32)
        nc.gpsimd.iota(iota_chunk[:], pattern=[[0, MAXC], [1, P]], base=0,
                       channel_multiplier=0, allow_small_or_imprecise_dtypes=True)
        node_iota = const.tile([P, 1], F32)
        nc.gpsimd.iota(node_iota[:], pattern=[[0, 1]], base=0,
                       channel_multiplier=1, allow_small_or_imprecise_dtypes=True)
        ident = const.tile([P, P], F32)
        make_identity(nc, ident[:])

        xe_own = own.tile([P, ntiles, 64], F32)
        exp_i = own.tile([P, ntiles, 4], F32R)  # [au_i | av_o] per own node
        exp_o = own.tile([P, ntiles, 4], F32R)  # [au_o | av_i]

        def load_const_r(t_w, shape):
            w = lhsp.tile(list(shape), F32, tag="wtmp", name="wtmp")
            nc.sync.dma_start(out=w[:], in_=t_w[: shape[0], : shape[1]])
            wr = const.tile(list(shape), F32R, name=f"wr_{t_w.name}")
            nc.scalar.mul(out=wr[:], in_=w[:], mul=1.0)
            return wr

        w_a1 = load_const_r(t_Wf_a1, (Dx + 1, 264))
        w_b1 = load_const_r(t_Wf_b1, (Dx + 1, 320))
        w_a2 = load_const_r(t_Wf_a2, (P, 264))
        w_b2 = load_const_r(t_Wf_b2, (P, 256))
        w_r = load_const_r(t_Wr, (Dg + 1, P))
        gTr = load_const_r(t_gT, (Dg + 1, R))

        def load_lhs(layer, col0, cn, src_own):
            cdim = Dx + 1 if layer == 1 else P
            lf = lhsp.tile([cdim, P], F32, tag="lhs_f")
            if layer == 1:
                src = t_xnT_own if src_own else t_xnT
                nc.sync.dma_start(out=lf[:, :cn], in_=src[:, col0 : col0 + cn])
            else:
                if src_own:
                    nc.sync.dma_start(out=lf[:, :cn],
                                      in_=d_hT_in[:, col0 : col0 + cn])
                else:
                    s = col0 // ns
                    j = col0 - s * ns
                    nc.sync.dma_start(
                        out=lf[:, :cn],
                        in_=d_hT[s * P : (s + 1) * P, j : j + cn])
            lr = lhsp.tile([cdim, P], F32R, tag="lhs_r")
            nc.scalar.mul(out=lr[:, :cn], in_=lf[:, :cn], mul=1.0)
            return lr

        def p0(layer):
            wa = w_a1 if layer == 1 else w_a2
            wb = w_b1 if layer == 1 else w_b2
            bcols = 320 if layer == 1 else 256
            for s in range(NCORES):
                for j0 in range(0, ntiles, 4):
                    jn = min(4, ntiles - j0)
                    st_i = stag.tile([P, 4, 132], F32, tag="st_i")
                    st_o = stag.tile([P, 4, 132], F32, tag="st_o")
                    for jj in range(jn):
                        j = j0 + jj
                        rn = min(P, ns - j * P)
                        lr = load_lhs(layer, s * ns + j * P, rn, False)
                        ps = psum.tile([P, 264], F32, space="PSUM", tag="p0ps", bufs=2)
                        nc.tensor.matmul(out=ps[:rn, :], lhsT=lr[:, :rn],
                                         rhs=wa[:], start=True, stop=True)
                        nc.scalar.copy(out=st_i[:rn, jj, :], in_=ps[:rn, 0:132])
                        nc.scalar.copy(out=st_o[:rn, jj, :], in_=ps[:rn, 132:264])
                    r0 = s * ns + j0 * P
                    rn_tot = min(4 * P, ns - j0 * P)
                    full, rem = rn_tot // P, rn_tot % P
                    for d_t, st in ((d_Ui, st_i), (d_Uo, st_o)):
                        if full:
                            nc.sync.dma_start(
                                out=d_t[r0 : r0 + full * P, 0:132].rearrange(
                                    "(c p) d -> p c d", p=P),
                                in_=st[:, :full, :])
                        if rem:
                            nc.sync.dma_start(
                                out=d_t[r0 + full * P : r0 + rn_tot, 0:132],
                                in_=st[:rem, full, :])
            if layer == 2:
                nc.sync.dma_start(out=d_Sav[:, 0:2], in_=d_Uo[:, 130:132])
            # own-shard: V tables (+ xe in L1) and expansion scalars
            for j in range(ntiles):
                rn = min(P, ns - j * P)
                lr = load_lhs(layer, j * P, rn, True)
                ps = psum.tile([P, bcols], F32, space="PSUM", tag="pmisc")
                nc.tensor.matmul(out=ps[:rn, :], lhsT=lr[:, :rn],
                                 rhs=wb[:, :bcols], start=True, stop=True)
                stv = stag.tile([P, 256], F32, tag="stv")
                nc.scalar.copy(out=stv[:rn, :], in_=ps[:rn, 0:256])
                nc.sync.dma_start(out=d_V[j * P : j * P + rn, :],
                                  in_=stv[:rn, :])
                if layer == 1:
                    nc.vector.tensor_copy(out=xe_own[:rn, j, :],
                                          in_=ps[:rn, 256:320])
                ps2 = psum.tile([P, 8], F32, space="PSUM", tag="pmisc2")
                nc.tensor.matmul(out=ps2[:rn, 0:4], lhsT=lr[:, :rn],
                                 rhs=wa[:, 128:132], start=True, stop=True)
                nc.tensor.matmul(out=ps2[:rn, 4:8], lhsT=lr[:, :rn],
                                 rhs=wa[:, 260:264], start=True, stop=True)
                nc.scalar.copy(out=exp_i[:rn, j, :], in_=ps2[:rn, 0:4])
                nc.scalar.copy(out=exp_o[:rn, j, :], in_=ps2[:rn, 4:8])

        def load_idx(order):
            eo = sa if order == "sa" else sb
            fa = idxp.tile([P, max(sa.S, sb.S) // 16], I16, tag="fa")
            ett = idxp.tile([P, max(sa.S, sb.S) // 16], I16, tag="et")
            kc = idxp.tile([P, max(sa.S, sb.S) // P], F32, tag="kc")
            nc.sync.dma_start(out=fa[:, : eo.S // 16], in_=t_idx["fa_" + order][:])
            nc.sync.dma_start(out=ett[:, : eo.S // 16], in_=t_idx["et_" + order][:])
            nc.sync.dma_start(out=kc[:, : eo.S // P], in_=t_idx["kc_" + order][:])
            return eo, fa, ett, kc

        def build_M(eo, kc, order, t):
            C = eo.n_tile[t] // P
            c0 = int(eo.off[t]) // P
            m_e = mpool.tile([P, MAXC, P], F32R, tag="m_e")
            nc.vector.tensor_tensor(
                out=m_e[:, :C, :], in0=iota_chunk[:, :C, :],
                in1=kc[:, c0 : c0 + C].to_broadcast([P, C, P]),
                op=ALU.is_equal)
            kr = small.tile([P, MAXC * P], F32, tag="kr")
            src_ap = t_idx["kr_" + order][:, int(eo.off[t]) : int(eo.off[t + 1])]
            nc.sync.dma_start(
                out=kr[:, : C * P],
                in_=bass.AP(src_ap.tensor, src_ap.offset,
                            [[0, P]] + src_ap.ap[1:]))
            m_n = mpool.tile([P, MAXC * P], F32R, tag="m_n")
            nc.vector.tensor_scalar(
                out=m_n[:, : C * P], in0=kr[:, : C * P],
                scalar1=node_iota[:], scalar2=None, op0=ALU.is_equal)
            return C, m_e, m_n

        def gather_pair(eo, fa, ett, t, table, ttable, width):
            nlo, nhi, ntl = eo.n_lo[t], eo.n_hi[t], eo.n_tile[t]
            off = int(eo.off[t])
            nrows = table.shape[0] if hasattr(table, "shape") else N
            lo_top = min(HI_SPLIT, nrows)
            hi_base = HI_SPLIT if nrows > HI_SPLIT else 0
            gU = gathp.tile([P, MAXC, width], F32, tag=f"gU{width}")

            def gcalls(dst, src_t, iarr, i0, num, c0):
                for q0 in range(0, num, 1024):
                    qn = min(1024, num - q0)
                    nc.gpsimd.dma_gather(
                        out_ap=dst[:, c0 + q0 // P : c0 + (q0 + qn) // P, :],
                        in_ap=src_t,
                        idxs_ap=iarr[:, (i0 + q0) // 16 : (i0 + q0 + qn) // 16],
                        num_idxs=qn, num_idxs_reg=qn,
                        elem_size=width, elem_step=width)

            gcalls(gU, table[:lo_top, :], fa, off, nlo, 0)
            gcalls(gU, table[hi_base:, :], fa, off + nlo, nhi, nlo // P)
            gT = gathp2.tile([P, MAXC, width], F32, tag=f"gT{width}")
            gcalls(gT, ttable[:], ett, off, ntl, 0)
            return gU, gT

        def expand(m_n, C, vals, t):
            ps = psagg.tile([P, MAXC, 4], F32, space="PSUM", tag="expps")
            for c in range(C):
                nc.tensor.matmul(
                    out=ps[:, c, :], lhsT=m_n[:, c * P : (c + 1) * P],
                    rhs=vals[:, t, :], start=True, stop=True)
            return ps

        def softmax_e(C, terms, out, out_sl):
            """out[:, :C, out_sl] = exp(lrelu(sum(terms)))"""
            a = small.tile([P, MAXC, 2], F32, tag="a_t")
            nc.vector.tensor_tensor(out=a[:, :C, :], in0=terms[0], in1=terms[1],
                                    op=ALU.add)
            nc.vector.tensor_tensor(out=a[:, :C, :], in0=a[:, :C, :],
                                    in1=terms[2], op=ALU.add)
            asc = small.tile([P, MAXC, 2], F32, tag="asc")
            nc.vector.tensor_scalar_mul(out=asc[:, :C, :], in0=a[:, :C, :],
                                        scalar1=NEG_SLOPE)
            nc.vector.tensor_tensor(out=a[:, :C, :], in0=a[:, :C, :],
                                    in1=asc[:, :C, :], op=ALU.max)
            nc.scalar.activation(out=out[:, :C, out_sl] if out_sl else out[:, :C, :],
                                 in_=a[:, :C, :], func=AF.Exp)

        def pass_dn(layer, direction):
            """dn_in over SA order (normalization key = key0)."""
            order = "sa"
            eo, fa, ett, kc = load_idx(order)
            savt = t_Sav1 if layer == 1 else d_Sav
            satt = t_Sat[layer]
            dn_st = own.tile([P, ntiles, 2], F32, tag="dn_st")
            for t in range(ntiles):
                C, m_e, m_n = build_M(eo, kc, order, t)
                gA, gAt = gather_pair(eo, fa, ett, t, savt, satt, 64)
                ex = expand(m_n, C, exp_i, t)  # au_i at cols 0:2
                e = small.tile([P, MAXC, 2], F32R, tag="e_dn")
                softmax_e(C, [ex[:, :C, 0:2], gA[:, :C, 0:2], gAt[:, :C, 0:2]],
                          e, None)
                psd = psagg.tile([P, 2], F32, space="PSUM", tag="dnp")
                for c in range(C):
                    nc.tensor.matmul(out=psd[:], lhsT=m_e[:, c, :],
                                     rhs=e[:, c, :], start=(c == 0),
                                     stop=(c == C - 1))
                nc.scalar.copy(out=dn_st[:, t, :], in_=psd[:])
            nc.vector.tensor_scalar(out=dn_st[:], in0=dn_st[:], scalar1=1e-35,
                                    scalar2=None, op0=ALU.max)
            nc.vector.reciprocal(out=dn_st[:], in_=dn_st[:])
            return dn_st

        def allgather_dn(dn_st, bufs, utable):
            d_in, d_out = bufs
            full, rem = ns // P, ns % P
            nc.sync.dma_start(
                out=d_in[: full * P, :].rearrange("(c p) d -> p c d", p=P),
                in_=dn_st[:, :full, :])
            if rem:
                nc.sync.dma_start(out=d_in[full * P :, :],
                                  in_=dn_st[:rem, full, :])
            nc.gpsimd.collective_compute(
                "AllGather", ALU.bypass, replica_groups=RG,
                ins=[d_in[:]], outs=[d_out[:]])
            nc.sync.dma_start(out=utable[:, 132:134], in_=d_out[:])

        def pass_agg(layer, order, direction, h_dst):
            eo, fa, ett, kc = load_idx(order)
            ut = d_Ui if direction == "i" else d_Uo
            tt = t_Tt[(layer, direction)]
            exp_tile = exp_o if direction == "i" else exp_i
            dn2 = (own.tile([P, ntiles, 2], F32, tag="dn2_st", name="dn2_st")
                   if order == "sb" else None)
            for t in range(ntiles):
                rn = min(P, ns - t * P)
                C, m_e, m_n = build_M(eo, kc, order, t)
                gU, gT = gather_pair(eo, fa, ett, t, ut, tt, NT)
                ex = expand(m_n, C, exp_tile, t)
                rhs = work.tile([P, MAXC, 130], F32R, tag="rhs")
                e1 = small.tile([P, MAXC, 2], F32, tag="e1")
                # this direction's e then alpha (into rhs cols 128:130)
                softmax_e(C, [gU[:, :C, 128:130], ex[:, :C, 2:4],
                              gT[:, :C, 128:130]], e1, None)
                nc.vector.tensor_tensor(out=rhs[:, :C, 128:130],
                                        in0=e1[:, :C, :],
                                        in1=gU[:, :C, 132:134], op=ALU.mult)
                if order == "sb":
                    e2 = small.tile([P, MAXC, 2], F32R, tag="e2")
                    softmax_e(C, [gU[:, :C, 130:132], ex[:, :C, 0:2],
                                  gT[:, :C, 130:132]], e2, None)
                nc.vector.tensor_tensor(out=rhs[:, :C, 0:128],
                                        in0=gU[:, :C, 0:128],
                                        in1=gT[:, :C, 0:128], op=ALU.add)
                ps_agg = psagg.tile([P, 130], F32, space="PSUM", tag="aggps", bufs=2)
                ps_dn = (psagg.tile([P, 2], F32, space="PSUM", tag="dnp",
                                    name="ps_dn")
                         if order == "sb" else None)
                for c in range(C):
                    for h in range(2):
                        nc.scalar.activation(
                            out=rhs[:, c, h * 64 : (h + 1) * 64],
                            in_=rhs[:, c, h * 64 : (h + 1) * 64],
                            func=AF.Copy, scale=rhs[:, c, 128 + h : 129 + h].bitcast(F32))
                    nc.tensor.matmul(out=ps_agg[:], lhsT=m_e[:, c, :],
                                     rhs=rhs[:, c, :], start=(c == 0),
                                     stop=(c == C - 1))
                    if order == "sb":
                        nc.tensor.matmul(out=ps_dn[:], lhsT=m_e[:, c, :],
                                         rhs=e2[:, c, :], start=(c == 0),
                                         stop=(c == C - 1))
                if order == "sb":
                    nc.scalar.copy(out=dn2[:, t, :], in_=ps_dn[:])
                vt = small.tile([P, 256], F32, tag="vt")
                nc.sync.dma_start(out=vt[:rn, :], in_=d_V[t * P : t * P + rn, :])
                sc = small.tile([P, 2], F32, tag="sc")
                nc.vector.tensor_copy(out=sc[:], in_=ps_agg[:, 128:130])
                voff = 0 if direction == "i" else 128
                ht = small.tile([P, P], F32, tag="ht")
                for h in range(2):
                    nc.vector.tensor_scalar(
                        out=ht[:, h * 64 : (h + 1) * 64],
                        in0=vt[:, voff + h * 64 : voff + (h + 1) * 64],
                        scalar1=sc[:, h : h + 1], scalar2=None, op0=ALU.mult)
                nc.vector.tensor_tensor(out=ht[:], in0=ht[:],
                                        in1=ps_agg[:, 0:128], op=ALU.add)
                nc.sync.dma_start(out=h_dst[t * P : t * P + rn, :],
                                  in_=ht[:rn, :])
            if order == "sb":
                nc.vector.tensor_scalar(out=dn2[:], in0=dn2[:], scalar1=1e-35,
                                        scalar2=None, op0=ALU.max)
                nc.vector.reciprocal(out=dn2[:], in_=dn2[:])
            return dn2

        def combine(layer):
            for t in range(ntiles):
                rn = min(P, ns - t * P)
                hi = small.tile([P, P], F32, tag="c_hi")
                ho = small.tile([P, P], F32, tag="c_ho")
                nc.sync.dma_start(out=hi[:rn], in_=d_hin[t * P : t * P + rn, :])
                nc.sync.dma_start(out=ho[:rn], in_=d_hout[t * P : t * P + rn, :])
                hs = small.tile([P, P], F32, tag="c_hs")
                nc.vector.tensor_tensor(out=hs[:], in0=hi[:], in1=ho[:],
                                        op=ALU.add)
                hm = small.tile([P, P], F32, tag="c_hm")
                nc.vector.tensor_scalar_mul(out=hm[:], in0=hs[:],
                                            scalar1=ALPHA_MIX * NEG_SLOPE)
                nc.vector.tensor_scalar_mul(out=hs[:], in0=hs[:],
                                            scalar1=ALPHA_MIX)
                nc.vector.tensor_tensor(out=hs[:], in0=hs[:], in1=hm[:],
                                        op=ALU.max)
                ssq = small.tile([P, 2], F32, tag="c_ssq")
                sq = small.tile([P, 64], F32, tag="c_sq")
                for h in range(2):
                    nc.scalar.activation(
                        out=sq[:], in_=hs[:, h * 64 : (h + 1) * 64],
                        func=AF.Square, accum_out=ssq[:, h : h + 1])
                nc.scalar.sqrt(out=ssq[:], in_=ssq[:])
                nc.vector.tensor_scalar(out=ssq[:], in0=ssq[:], scalar1=EPS,
                                        scalar2=None, op0=ALU.max)
                nc.vector.reciprocal(out=ssq[:], in_=ssq[:])
                for h in range(2):
                    nc.vector.tensor_scalar(
                        out=hs[:, h * 64 : (h + 1) * 64],
                        in0=hs[:, h * 64 : (h + 1) * 64],
                        scalar1=ssq[:, h : h + 1], scalar2=None, op0=ALU.mult)
                if layer == 1:
                    pst = psum.tile([P, P], F32, space="PSUM", tag="pmisc")
                    nc.tensor.transpose(out=pst[:], in_=hs[:], identity=ident[:])
                    hT = small.tile([P, P], F32, tag="c_hT")
                    nc.vector.tensor_copy(out=hT[:], in_=pst[:])
                    nc.sync.dma_start(out=d_hT_in[:, t * P : t * P + rn],
                                      in_=hT[:, :rn])
                else:
                    hp = small.tile([P, P], F32, tag="c_hp")
                    for h in range(2):
                        nc.vector.tensor_tensor(
                            out=hp[:, h * 64 : (h + 1) * 64],
                            in0=hs[:, h * 64 : (h + 1) * 64],
                            in1=xe_own[:, t, :], op=ALU.add)
                    ssq2 = small.tile([P, 1], F32, tag="c_ssq2")
                    sq2 = small.tile([P, P], F32, tag="c_sq2")
                    nc.scalar.activation(out=sq2[:], in_=hp[:], func=AF.Square,
                                         accum_out=ssq2[:])
                    nc.scalar.sqrt(out=ssq2[:], in_=ssq2[:])
                    nc.vector.tensor_scalar(out=ssq2[:], in0=ssq2[:],
                                            scalar1=EPS, scalar2=None,
                                            op0=ALU.max)
                    nc.vector.reciprocal(out=ssq2[:], in_=ssq2[:])
                    nc.vector.tensor_scalar_mul(out=hp[:], in0=hp[:],
                                                scalar1=ssq2[:])
                    nc.sync.dma_start(out=o_h[t * P : t * P + rn, :],
                                      in_=hp[:rn, :])

        for layer in (1, 2):
            p0(layer)
            tc.strict_bb_all_engine_barrier()
            dn_i = pass_dn(layer, "i")
            allgather_dn(dn_i, d_dn[(layer, "i")], d_Ui)
            tc.strict_bb_all_engine_barrier()
            dn_o = pass_agg(layer, "sb", "i", d_hin)
            allgather_dn(dn_o, d_dn[(layer, "o")], d_Uo)
            tc.strict_bb_all_engine_barrier()
            pass_agg(layer, "sa", "o", d_hout)
            tc.strict_bb_all_engine_barrier()
            combine(layer)
            if layer == 1:
                nc.gpsimd.collective_compute(
                    "AllGather", ALU.bypass, replica_groups=RG,
                    ins=[d_hT_in[:]], outs=[d_hT[:]])
                tc.strict_bb_all_engine_barrier()

        for c0 in range(0, R, P):
            cn = min(P, R - c0)
            psg = psum.tile([P, P], F32, space="PSUM", tag="pmisc")
            nc.tensor.matmul(out=psg[:cn, :], lhsT=gTr[:, c0 : c0 + cn],
                             rhs=w_r[:], start=True, stop=True)
            gpt = small.tile([P, P], F32, tag="gpt")
            nc.scalar.copy(out=gpt[:cn, :], in_=psg[:cn, :])
            nc.sync.dma_start(out=o_gp[c0 : c0 + cn, :], in_=gpt[:cn, :])

    nc.compile()
    if split:
        split_excess_waits(nc, max_waits=1)
    return nc


_CACHE = {}


def kernel(x, g, edge_idx, edge_type, W1i, b1i, a1i, W1o, b1o, a1o,
           W2i, b2i, a2i, W2o, b2o, a2o, We, be, Wr, br):
    x = np.asarray(x, np.float32)
    g = np.asarray(g, np.float32)
    W = {k: np.asarray(v, np.float32) for k, v in dict(
        W1i=W1i, b1i=b1i, a1i=a1i, W1o=W1o, b1o=b1o, a1o=a1o,
        W2i=W2i, b2i=b2i, a2i=a2i, W2o=W2o, b2o=b2o, a2o=a2o,
        We=We, be=be, Wr=Wr, br=br).items()}
    meta, in_maps = _prep_host(x, g, edge_idx, edge_type, W)
    ckey = (x.shape, g.shape, meta["sa"].S, meta["sb"].S)
    if ckey not in _CACHE:
        _CACHE[ckey] = _build_nc(meta)
    res = run_bass_kernel_spmd(_CACHE[ckey], in_maps,
                               core_ids=list(range(NCORES)))
    global LAST_RESULTS
    LAST_RESULTS = res
    hp = np.concatenate([res.results[k]["hprime"] for k in range(NCORES)], 0)
    gp = res.results[0]["gprime"]
    return hp, gp


LAST_RESULTS = None
